# revision 55
# baseline (speedup 1.0000x reference)
"""Trainium2 Bass kernel: MoE actor-critic (8 experts) over 8 NeuronCores.

Strategy: pure data-parallel on batch B=16384 -> 2048 rows/core, weights
replicated. Per core, one fused Tile program computes:
  - 8 expert policy nets (bf16): actions[E, 2048, 32] = tanh(...)
  - gating net in split-precision bf16x3 (fp32-accurate, bf16-rate):
    x@W = xh@Wh + xh@Wl + xl@Wh with h/l bf16 splits; softmax -> w
  - gumbel-argmax mask from ln(w) + g (host-precomputed gumbel constant)
  - 8 expert value nets (bf16 matmul, fp32 accum) -> vals_b[128, NT, E]
  - v = sum_e mask_e * vals_e
Activations are feature-major [feat(part), batch(free)] so weights are used
as lhsT directly with no transposes; last layers are batch-stationary.
"""

import os
import sys

import numpy as np

for _p in ("/opt/trn_rl_repo", "/root/.axon_site/_ro/trn_rl_repo"):
    if os.path.isdir(_p) and _p not in sys.path:
        sys.path.insert(0, _p)

import ml_dtypes  # noqa: E402
import concourse.bacc as bacc  # noqa: E402
import concourse.mybir as mybir  # noqa: E402
import concourse.tile as tile  # noqa: E402
from concourse.bass_utils import run_bass_kernel_spmd  # noqa: E402

F32 = mybir.dt.float32
BF16 = mybir.dt.bfloat16
NPBF16 = ml_dtypes.bfloat16
AF = mybir.ActivationFunctionType
ALU = mybir.AluOpType
AX = mybir.AxisListType

B, D, H, HV, A, E = 16384, 512, 512, 256, 32, 8
NCORES = 8
BL = B // NCORES      # 2048 rows per core
NT = BL // 128        # 16 batch tiles of 128 rows
MC = BL // 512        # 4 chunks of 512 batch cols (matmul free dim)
KD = D // 128         # 4 k-subtiles for 512-wide contractions
KH = HV // 128        # 2 k-subtiles for 256-wide contractions

LAST_EXEC_NS = None
_CACHED_NC = None


def _build():
    nc = bacc.Bacc()

    # ---------------- DRAM parameters (per-core shapes) ----------------
    # m-major pre-tiled x: [p, m, k, c] so each 512-col chunk is contiguous
    xtb = nc.declare_dram_parameter("xtb", [128, MC, KD, 512], BF16, False)
    xlo = nc.declare_dram_parameter("xlo", [128, MC, KD, 512], BF16, False)
    # n-major policy l1 weights: [e, p, n, k, c] = W[e, k*128+p, n*128+c]
    wp0 = nc.declare_dram_parameter("wp0", [E, 128, KD, KD, 128], BF16, False)
    wp1 = nc.declare_dram_parameter("wp1", [E, 128, KD, H], BF16, False)
    wmu = nc.declare_dram_parameter("wmu", [E, 128, KD, A], BF16, False)
    bp0 = nc.declare_dram_parameter("bp0", [E, 128, KD], F32, False)
    bp1 = nc.declare_dram_parameter("bp1", [E, 128, KD], F32, False)
    bmu = nc.declare_dram_parameter("bmu", [E, 128, A], F32, False)
    wv0 = nc.declare_dram_parameter("wv0", [E, 128, KD, HV], BF16, False)
    wv1 = nc.declare_dram_parameter("wv1", [E, 128, KH, HV], BF16, False)
    wv = nc.declare_dram_parameter("wv", [E, 128, KH], BF16, False)
    bv0 = nc.declare_dram_parameter("bv0", [E, 128, KH], F32, False)
    bv1 = nc.declare_dram_parameter("bv1", [E, 128, KH], F32, False)
    bvb = nc.declare_dram_parameter("bvb", [128, E], F32, False)
    ww0h = nc.declare_dram_parameter("ww0h", [128, KD, H], BF16, False)
    ww0l = nc.declare_dram_parameter("ww0l", [128, KD, H], BF16, False)
    ww1h = nc.declare_dram_parameter("ww1h", [128, KD, H], BF16, False)
    ww1l = nc.declare_dram_parameter("ww1l", [128, KD, H], BF16, False)
    wwoh = nc.declare_dram_parameter("wwoh", [128, KD, E], BF16, False)
    wwol = nc.declare_dram_parameter("wwol", [128, KD, E], BF16, False)
    bw0 = nc.declare_dram_parameter("bw0", [128, KD], F32, False)
    bw1 = nc.declare_dram_parameter("bw1", [128, KD], F32, False)
    bwo = nc.declare_dram_parameter("bwo", [128, E], F32, False)
    gmb = nc.declare_dram_parameter("gmb", [128, NT, E], F32, False)
    out_act = nc.declare_dram_parameter("actions", [E, NT, 128, A], F32, True)
    out_v = nc.declare_dram_parameter("v", [128, NT], F32, True)
    out_w = nc.declare_dram_parameter("w", [128, NT, E], F32, True)

    with tile.TileContext(nc) as tc:
        with (
            tc.tile_pool(name="persist", bufs=1) as persist,
            tc.tile_pool(name="xpool", bufs=1) as xpool,
            tc.tile_pool(name="gw", bufs=1) as gw,
            tc.tile_pool(name="gact", bufs=1) as gact,
            tc.tile_pool(name="ew", bufs=2) as ew,
            tc.tile_pool(name="eact", bufs=1) as eact,
            tc.tile_pool(name="eact2", bufs=2) as eact2,
            tc.tile_pool(name="stage", bufs=4) as stage,
            tc.tile_pool(name="smalls", bufs=4) as smalls,
            tc.tile_pool(name="pbig", bufs=5, space="PSUM") as pbig,
            tc.tile_pool(name="psmall", bufs=3, space="PSUM") as psmall,
        ):
            # x (bf16) first: the policy experts start as soon as the first
            # 512-column chunk lands; first-needed DMAs issued from separate
            # engines so descriptor generation runs in parallel
            xtb_sb = xpool.tile([128, MC, KD, 512], BF16)
            nc.sync.dma_start(xtb_sb[:, 0], xtb[:, 0])

            def load_policy_weights(e):
                eng = nc.gpsimd if e == 0 else nc.sync
                wp0_sb = ew.tile([128, KD, KD, 128], BF16, tag="wp0", name="wp0_sb")
                if e == 0:
                    for n in range(KD):
                        eng.dma_start(wp0_sb[:, n], wp0[e, :, n])
                else:
                    eng.dma_start(wp0_sb[:], wp0[e])
                bp0_sb = ew.tile([128, KD], F32, tag="bp0", name="bp0_sb")
                eng.dma_start(bp0_sb[:], bp0[e])
                if e == 0:
                    # first-needed data is in flight; queue the rest behind it
                    for m in range(1, MC):
                        nc.sync.dma_start(xtb_sb[:, m], xtb[:, m])
                    eng = nc.scalar
                wp1_sb = ew.tile([128, KD, H], BF16, tag="wp1", name="wp1_sb")
                eng.dma_start(wp1_sb[:], wp1[e])
                wmu_sb = ew.tile([128, KD, A], BF16, tag="wmu", name="wmu_sb")
                eng.dma_start(wmu_sb[:], wmu[e])
                bp1_sb = ew.tile([128, KD], F32, tag="bp1", name="bp1_sb")
                eng.dma_start(bp1_sb[:], bp1[e])
                bmu_sb = ew.tile([128, A], F32, tag="bmu", name="bmu_sb")
                eng.dma_start(bmu_sb[:], bmu[e])
                return wp0_sb, wp1_sb, wmu_sb, bp0_sb, bp1_sb, bmu_sb

            pol_w0 = load_policy_weights(0)

            # aux + gating tiles (DMAs deferred into the policy loop so the
            # first expert's loads get the full DMA bandwidth)
            gumbel_sb = persist.tile([128, NT, E], F32)
            w_sb = persist.tile([128, NT, E], F32)
            mask_sb = persist.tile([128, NT, E], F32)
            vals_b = persist.tile([128, NT, E], F32)
            v_sb = persist.tile([128, NT], F32)
            bwo_bc = persist.tile([128, E], F32)
            bvb_bc = persist.tile([128, E], F32)
            xlo_sb = xpool.tile([128, MC, KD, 512], BF16)
            ww0h_sb = gw.tile([128, KD, H], BF16)
            ww0l_sb = gw.tile([128, KD, H], BF16)
            ww1h_sb = gw.tile([128, KD, H], BF16)
            ww1l_sb = gw.tile([128, KD, H], BF16)
            wwoh_sb = gw.tile([128, KD, E], BF16)
            wwol_sb = gw.tile([128, KD, E], BF16)
            bw0_sb = gw.tile([128, KD], F32)
            bw1_sb = gw.tile([128, KD], F32)

            def load_aux():
                nc.sync.dma_start(gumbel_sb[:], gmb[:])
                nc.sync.dma_start(bwo_bc[:], bwo[:])
                nc.sync.dma_start(bvb_bc[:], bvb[:])
                nc.sync.dma_start(xlo_sb[:], xlo[:])
                nc.sync.dma_start(ww0h_sb[:], ww0h[:])
                nc.sync.dma_start(ww0l_sb[:], ww0l[:])
                nc.sync.dma_start(ww1h_sb[:], ww1h[:])
                nc.sync.dma_start(ww1l_sb[:], ww1l[:])
                nc.sync.dma_start(wwoh_sb[:], wwoh[:])
                nc.sync.dma_start(wwol_sb[:], wwol[:])
                nc.sync.dma_start(bw0_sb[:], bw0[:])
                nc.sync.dma_start(bw1_sb[:], bw1[:])

            # ---------------- expert policy nets (bf16) ----------------
            def policy_expert(e, weights):
                wp0_sb, wp1_sb, wmu_sb, bp0_sb, bp1_sb, bmu_sb = weights
                h1 = eact2.tile([128, KD, BL], BF16, tag="h1", name="h1")
                for n in range(KD):
                    for m in range(MC):
                        ps = pbig.tile([128, 512], F32, tag="big", name="ps")
                        for k in range(KD):
                            nc.tensor.matmul(
                                ps[:],
                                wp0_sb[:, n, k, :],
                                xtb_sb[:, m, k, :],
                                start=(k == 0),
                                stop=(k == KD - 1),
                            )
                        nc.scalar.activation(
                            h1[:, n, m * 512 : (m + 1) * 512],
                            ps[:],
                            AF.Relu,
                            bias=bp0_sb[:, n : n + 1],
                        )
                h2 = eact.tile([128, KD, BL], BF16, tag="h2", name="h2")
                for n in range(KD):
                    for m in range(MC):
                        ps = pbig.tile([128, 512], F32, tag="big", name="ps")
                        for k in range(KD):
                            nc.tensor.matmul(
                                ps[:],
                                wp1_sb[:, k, n * 128 : (n + 1) * 128],
                                h1[:, k, m * 512 : (m + 1) * 512],
                                start=(k == 0),
                                stop=(k == KD - 1),
                            )
                        nc.scalar.activation(
                            h2[:, n, m * 512 : (m + 1) * 512],
                            ps[:],
                            AF.Relu,
                            bias=bp1_sb[:, n : n + 1],
                        )
                # policy head, batch-stationary: [128, A] per batch tile
                for t in range(NT):
                    psA = psmall.tile([128, A], F32, tag="small", name="psA")
                    for k in range(KD):
                        nc.tensor.matmul(
                            psA[:],
                            h2[:, k, t * 128 : (t + 1) * 128],
                            wmu_sb[:, k, :],
                            start=(k == 0),
                            stop=(k == KD - 1),
                        )
                    a_pre = smalls.tile([128, A], F32, tag="a_pre", name="a_pre")
                    nc.vector.tensor_add(a_pre[:], psA[:], bmu_sb[:])
                    a_st = stage.tile([128, A], F32, tag="a_st", name="a_st")
                    nc.scalar.activation(a_st[:], a_pre[:], AF.Tanh)
                    nc.sync.dma_start(out_act[e, t], a_st[:])

            policy_expert(0, pol_w0)
            for e in range(1, E):
                w_e = load_policy_weights(e)
                if e == 1:
                    load_aux()
                policy_expert(e, w_e)

            # ---------------- expert value nets (bf16) ----------------
            for e in range(E):
                wv0_sb = ew.tile([128, KD, HV], BF16, tag="wv0", name="wv0_sb")
                nc.sync.dma_start(wv0_sb[:], wv0[e])
                wv1_sb = ew.tile([128, KH, HV], BF16, tag="wv1", name="wv1_sb")
                nc.sync.dma_start(wv1_sb[:], wv1[e])
                wv_sb = ew.tile([128, KH], BF16, tag="wv", name="wv_sb")
                nc.sync.dma_start(wv_sb[:], wv[e])
                bv0_sb = ew.tile([128, KH], F32, tag="bv0", name="bv0_sb")
                nc.sync.dma_start(bv0_sb[:], bv0[e])
                bv1_sb = ew.tile([128, KH], F32, tag="bv1", name="bv1_sb")
                nc.sync.dma_start(bv1_sb[:], bv1[e])

                g1 = eact2.tile([128, KH, BL], BF16, tag="g1", name="g1")
                for n in range(KH):
                    for m in range(MC):
                        ps = pbig.tile([128, 512], F32, tag="big", name="ps")
                        for k in range(KD):
                            nc.tensor.matmul(
                                ps[:],
                                wv0_sb[:, k, n * 128 : (n + 1) * 128],
                                xtb_sb[:, m, k, :],
                                start=(k == 0),
                                stop=(k == KD - 1),
                            )
                        if (n * MC + m) % 2 == 0:
                            nc.vector.tensor_scalar(
                                g1[:, n, m * 512 : (m + 1) * 512],
                                ps[:],
                                bv0_sb[:, n : n + 1],
                                0.0,
                                ALU.add,
                                op1=ALU.max,
                            )
                        else:
                            nc.scalar.activation(
                                g1[:, n, m * 512 : (m + 1) * 512],
                                ps[:],
                                AF.Relu,
                                bias=bv0_sb[:, n : n + 1],
                            )
                g2 = eact.tile([128, KH, BL], BF16, tag="g2", name="g2")
                for n in range(KH):
                    for m in range(MC):
                        ps = pbig.tile([128, 512], F32, tag="big", name="ps")
                        for k in range(KH):
                            nc.tensor.matmul(
                                ps[:],
                                wv1_sb[:, k, n * 128 : (n + 1) * 128],
                                g1[:, k, m * 512 : (m + 1) * 512],
                                start=(k == 0),
                                stop=(k == KH - 1),
                            )
                        if (n * MC + m) % 2 == 0:
                            nc.vector.tensor_scalar(
                                g2[:, n, m * 512 : (m + 1) * 512],
                                ps[:],
                                bv1_sb[:, n : n + 1],
                                0.0,
                                ALU.add,
                                op1=ALU.max,
                            )
                        else:
                            nc.scalar.activation(
                                g2[:, n, m * 512 : (m + 1) * 512],
                                ps[:],
                                AF.Relu,
                                bias=bv1_sb[:, n : n + 1],
                            )
                # value head, batch-stationary: [128, 1] per batch tile
                for t in range(NT):
                    psv = psmall.tile([128, 1], F32, tag="small", name="psv")
                    for k in range(KH):
                        nc.tensor.matmul(
                            psv[:],
                            g2[:, k, t * 128 : (t + 1) * 128],
                            wv_sb[:, k : k + 1],
                            start=(k == 0),
                            stop=(k == KH - 1),
                        )
                    nc.vector.tensor_scalar_add(
                        vals_b[:, t, e : e + 1], psv[:], bvb_bc[:, e : e + 1]
                    )

            # ---------------- gating network (bf16x3 split precision) ----
            for mb in range(MC):
                ms = mb * 512
                u1 = gact.tile([128, KD, 512], F32, tag="u1", name="u1")
                for n in range(KD):
                    ps = pbig.tile([128, 512], F32, tag="big", name="ps")
                    nt_sl = slice(n * 128, (n + 1) * 128)
                    for k in range(KD):
                        nc.tensor.matmul(
                            ps[:], ww0h_sb[:, k, nt_sl],
                            xtb_sb[:, mb, k, :],
                            start=(k == 0), stop=False,
                        )
                    for k in range(KD):
                        nc.tensor.matmul(
                            ps[:], ww0l_sb[:, k, nt_sl],
                            xtb_sb[:, mb, k, :],
                            start=False, stop=False,
                        )
                    for k in range(KD):
                        nc.tensor.matmul(
                            ps[:], ww0h_sb[:, k, nt_sl],
                            xlo_sb[:, mb, k, :],
                            start=False, stop=(k == KD - 1),
                        )
                    nc.scalar.activation(
                        u1[:, n, :], ps[:], AF.Relu, bias=bw0_sb[:, n : n + 1]
                    )
                    if n == 0:
                        u1h = gact.tile([128, KD, 512], BF16, tag="u1h", name="u1h")
                        u1l = gact.tile([128, KD, 512], BF16, tag="u1l", name="u1l")
                    nc.vector.tensor_copy(u1h[:, n, :], u1[:, n, :])
                    nc.vector.tensor_sub(u1l[:, n, :], u1[:, n, :], u1h[:, n, :])
                u2 = gact.tile([128, KD, 512], F32, tag="u2", name="u2")
                for n in range(KD):
                    ps = pbig.tile([128, 512], F32, tag="big", name="ps")
                    nt_sl = slice(n * 128, (n + 1) * 128)
                    for k in range(KD):
                        nc.tensor.matmul(
                            ps[:], ww1h_sb[:, k, nt_sl], u1h[:, k, :],
                            start=(k == 0), stop=False,
                        )
                    for k in range(KD):
                        nc.tensor.matmul(
                            ps[:], ww1l_sb[:, k, nt_sl], u1h[:, k, :],
                            start=False, stop=False,
                        )
                    for k in range(KD):
                        nc.tensor.matmul(
                            ps[:], ww1h_sb[:, k, nt_sl], u1l[:, k, :],
                            start=False, stop=(k == KD - 1),
                        )
                    nc.scalar.activation(
                        u2[:, n, :], ps[:], AF.Relu, bias=bw1_sb[:, n : n + 1]
                    )
                    if n == 0:
                        u2h = gact.tile([128, KD, 512], BF16, tag="u2h", name="u2h")
                        u2l = gact.tile([128, KD, 512], BF16, tag="u2l", name="u2l")
                    nc.vector.tensor_copy(u2h[:, n, :], u2[:, n, :])
                    nc.vector.tensor_sub(u2l[:, n, :], u2[:, n, :], u2h[:, n, :])
                # logits (bf16x3) + softmax per 128-row tile
                for bt in range(4):
                    t = mb * 4 + bt
                    bt_sl = slice(bt * 128, (bt + 1) * 128)
                    ps8 = psmall.tile([128, E], F32, tag="small", name="ps8")
                    for k in range(KD):
                        nc.tensor.matmul(
                            ps8[:], u2h[:, k, bt_sl], wwoh_sb[:, k, :],
                            start=(k == 0), stop=False,
                        )
                    for k in range(KD):
                        nc.tensor.matmul(
                            ps8[:], u2h[:, k, bt_sl], wwol_sb[:, k, :],
                            start=False, stop=False,
                        )
                    for k in range(KD):
                        nc.tensor.matmul(
                            ps8[:], u2l[:, k, bt_sl], wwoh_sb[:, k, :],
                            start=False, stop=(k == KD - 1),
                        )
                    zl = smalls.tile([128, E], F32, tag="zl", name="zl")
                    nc.vector.tensor_add(zl[:], ps8[:], bwo_bc[:])
                    mx = smalls.tile([128, 1], F32, tag="mx", name="mx")
                    nc.vector.reduce_max(mx[:], zl[:], axis=AX.X)
                    negmx = smalls.tile([128, 1], F32, tag="negmx", name="negmx")
                    nc.vector.tensor_scalar_mul(negmx[:], mx[:], -1.0)
                    we = smalls.tile([128, E], F32, tag="we", name="we")
                    nc.scalar.activation(we[:], zl[:], AF.Exp, bias=negmx[:])
                    sm = smalls.tile([128, 1], F32, tag="sm", name="sm")
                    nc.vector.reduce_sum(sm[:], we[:], axis=AX.X)
                    rc = smalls.tile([128, 1], F32, tag="rc", name="rc")
                    nc.vector.reciprocal(rc[:], sm[:])
                    nc.vector.tensor_scalar_mul(w_sb[:, t, :], we[:], rc[:])
                    zt = smalls.tile([128, E], F32, tag="zt", name="zt")
                    nc.scalar.activation(zt[:], w_sb[:, t, :], AF.Ln)
                    nc.vector.tensor_add(zt[:], zt[:], gumbel_sb[:, t, :])
                    mxz = smalls.tile([128, 1], F32, tag="mxz", name="mxz")
                    nc.vector.reduce_max(mxz[:], zt[:], axis=AX.X)
                    nc.vector.tensor_scalar(
                        mask_sb[:, t, :], zt[:], mxz[:], None, ALU.is_ge
                    )
            nc.sync.dma_start(out_w[:], w_sb[:])
            # ---------------- select v = sum_e mask_e * vals_e ------------
            vsel = persist.tile([128, NT, E], F32)
            nc.vector.tensor_mul(vsel[:], mask_sb[:], vals_b[:])
            nc.vector.reduce_sum(v_sb[:], vsel[:], axis=AX.X)
            nc.sync.dma_start(out_v[:], v_sb[:])

    nc.compile()
    return nc


# ------------------------- host-side wrapper -------------------------


def _prep_w(wmat, ks, npdt):
    # [K, N] -> [p, ks, n] with K = ks*128
    n = wmat.shape[1]
    return np.ascontiguousarray(
        wmat.reshape(ks, 128, n).transpose(1, 0, 2)
    ).astype(npdt)


def _prep_b(bvec, ks):
    return np.ascontiguousarray(bvec.reshape(ks, 128).T).astype(np.float32)


def _bcast(bvec):
    return np.ascontiguousarray(
        np.broadcast_to(np.asarray(bvec, dtype=np.float32)[None, :], (128, len(bvec)))
    )


def _gumbel():
    import jax
    import jax.numpy as jnp

    try:
        dev = jax.devices("cpu")[0]
        with jax.default_device(dev):
            g = jax.random.gumbel(jax.random.key(42), (B, E), jnp.float32)
            return np.asarray(g)
    except Exception:
        g = jax.random.gumbel(jax.random.key(42), (B, E), jnp.float32)
        return np.asarray(g)


def _split_hi_lo(w):
    w = np.asarray(w, dtype=np.float32)
    hi = w.astype(NPBF16)
    lo = (w - hi.astype(np.float32)).astype(NPBF16)
    return hi, lo


def kernel(x, Wp0, bp0, Wp1, bp1, Wmu, bmu, Wv0, bv0, Wv1, bv1, Wv, bv,
           Ww0, bw0, Ww1, bw1, Wwo, bwo):
    global _CACHED_NC, LAST_EXEC_NS

    x = np.asarray(x, dtype=np.float32)
    gumbel = _gumbel()

    ww0h, ww0l = _split_hi_lo(Ww0)
    ww1h, ww1l = _split_hi_lo(Ww1)
    wwoh_, wwol_ = _split_hi_lo(Wwo)

    common = {
        "wp0": np.stack(
            [
                np.ascontiguousarray(
                    np.asarray(Wp0[e], dtype=np.float32)
                    .reshape(KD, 128, KD, 128)
                    .transpose(1, 2, 0, 3)
                ).astype(NPBF16)
                for e in range(E)
            ]
        ),
        "wp1": np.stack([_prep_w(np.asarray(Wp1[e]), KD, NPBF16) for e in range(E)]),
        "wmu": np.stack([_prep_w(np.asarray(Wmu[e]), KD, NPBF16) for e in range(E)]),
        "bp0": np.stack([_prep_b(np.asarray(bp0[e]), KD) for e in range(E)]),
        "bp1": np.stack([_prep_b(np.asarray(bp1[e]), KD) for e in range(E)]),
        "bmu": np.ascontiguousarray(
            np.broadcast_to(np.asarray(bmu, dtype=np.float32)[:, None, :], (E, 128, A))
        ),
        "wv0": np.stack([_prep_w(np.asarray(Wv0[e]), KD, NPBF16) for e in range(E)]),
        "wv1": np.stack([_prep_w(np.asarray(Wv1[e]), KH, NPBF16) for e in range(E)]),
        "wv": np.stack(
            [_prep_w(np.asarray(Wv[e]), KH, NPBF16)[:, :, 0] for e in range(E)]
        ),
        "bv0": np.stack([_prep_b(np.asarray(bv0[e]), KH) for e in range(E)]),
        "bv1": np.stack([_prep_b(np.asarray(bv1[e]), KH) for e in range(E)]),
        "bvb": _bcast(np.asarray(bv, dtype=np.float32).reshape(E)),
        "ww0h": _prep_w(ww0h.astype(np.float32), KD, NPBF16),
        "ww0l": _prep_w(ww0l.astype(np.float32), KD, NPBF16),
        "ww1h": _prep_w(ww1h.astype(np.float32), KD, NPBF16),
        "ww1l": _prep_w(ww1l.astype(np.float32), KD, NPBF16),
        "wwoh": _prep_w(wwoh_.astype(np.float32), KD, NPBF16),
        "wwol": _prep_w(wwol_.astype(np.float32), KD, NPBF16),
        "bw0": _prep_b(np.asarray(bw0), KD),
        "bw1": _prep_b(np.asarray(bw1), KD),
        "bwo": _bcast(np.asarray(bwo, dtype=np.float32)),
    }

    in_maps = []
    for c in range(NCORES):
        xs = x[c * BL : (c + 1) * BL]  # [BL, D]
        xT = np.ascontiguousarray(xs.T)  # [D, BL]
        # [p, m, k, c] = xT[k*128+p, m*512+c]
        xtf = np.ascontiguousarray(
            xT.reshape(KD, 128, MC, 512).transpose(1, 2, 0, 3)
        ).astype(np.float32)
        xhi = xtf.astype(NPBF16)
        xlo_ = (xtf - xhi.astype(np.float32)).astype(NPBF16)
        gs = gumbel[c * BL : (c + 1) * BL]  # [BL, E]
        gtile = np.ascontiguousarray(
            gs.reshape(NT, 128, E).transpose(1, 0, 2)
        ).astype(np.float32)
        m = dict(common)
        m["xtb"] = xhi
        m["xlo"] = xlo_
        m["gmb"] = gtile
        in_maps.append(m)

    if _CACHED_NC is None:
        _CACHED_NC = _build()
    nc = _CACHED_NC

    trace = os.environ.get("KERNEL_TRACE", "0") == "1"
    if trace:
        import concourse.bass_utils as _bu

        _bu.upload_artifacts = lambda d: d
        try:
            import antenv.axon_hooks  # noqa: F401
        except ImportError:
            import types

            import antenv
            from trn_agent_boot.trn_boot import _ntff_profile_via_ctypes

            _hook = _ntff_profile_via_ctypes("/opt/axon/libaxon_pjrt.so")
            mod = types.ModuleType("antenv.axon_hooks")
            mod._hook = _hook
            mod.get_axon_ntff_profile_hook = lambda: mod._hook
            mod.set_axon_ntff_profile_hook = lambda h: setattr(mod, "_hook", h)
            sys.modules["antenv.axon_hooks"] = mod
            antenv.axon_hooks = mod

    res = run_bass_kernel_spmd(nc, in_maps, list(range(NCORES)), trace=trace)
    LAST_EXEC_NS = res.exec_time_ns

    act_parts, v_parts, w_parts = [], [], []
    for c in range(NCORES):
        r = res.results[c]
        act_parts.append(np.asarray(r["actions"]).reshape(E, BL, A))
        v_parts.append(np.asarray(r["v"]).T.reshape(BL, 1))
        w_parts.append(np.asarray(r["w"]).transpose(1, 0, 2).reshape(BL, E))
    actions = np.concatenate(act_parts, axis=1)
    v = np.concatenate(v_parts, axis=0)
    w = np.concatenate(w_parts, axis=0)
    return actions, v, w


# revision 58
# speedup vs baseline: 1.0543x; 1.0543x over previous
"""Trainium2 Bass kernel: MoE actor-critic (8 experts) over 8 NeuronCores.

Strategy: pure data-parallel on batch B=16384 -> 2048 rows/core, weights
replicated. Per core, one fused Tile program computes:
  - 8 expert policy nets (bf16): actions[E, 2048, 32] = tanh(...)
  - gating net in split-precision bf16x3 (fp32-accurate, bf16-rate):
    x@W = xh@Wh + xh@Wl + xl@Wh with h/l bf16 splits; softmax -> w
  - gumbel-argmax mask from ln(w) + g (host-precomputed gumbel constant)
  - 8 expert value nets (bf16 matmul, fp32 accum) -> vals_b[128, NT, E]
  - v = sum_e mask_e * vals_e
Activations are feature-major [feat(part), batch(free)] so weights are used
as lhsT directly with no transposes; last layers are batch-stationary.
"""

import os
import sys

import numpy as np

for _p in ("/opt/trn_rl_repo", "/root/.axon_site/_ro/trn_rl_repo"):
    if os.path.isdir(_p) and _p not in sys.path:
        sys.path.insert(0, _p)

import ml_dtypes  # noqa: E402
import concourse.bacc as bacc  # noqa: E402
import concourse.mybir as mybir  # noqa: E402
import concourse.tile as tile  # noqa: E402
from concourse.bass_utils import run_bass_kernel_spmd  # noqa: E402

F32 = mybir.dt.float32
BF16 = mybir.dt.bfloat16
NPBF16 = ml_dtypes.bfloat16
AF = mybir.ActivationFunctionType
ALU = mybir.AluOpType
AX = mybir.AxisListType

B, D, H, HV, A, E = 16384, 512, 512, 256, 32, 8
NCORES = 8
BL = B // NCORES      # 2048 rows per core
NT = BL // 128        # 16 batch tiles of 128 rows
MC = BL // 512        # 4 chunks of 512 batch cols (matmul free dim)
KD = D // 128         # 4 k-subtiles for 512-wide contractions
KH = HV // 128        # 2 k-subtiles for 256-wide contractions

LAST_EXEC_NS = None
_CACHED_NC = None


def _build():
    nc = bacc.Bacc()

    # ---------------- DRAM parameters (per-core shapes) ----------------
    # m-major pre-tiled x: [p, m, k, c] so each 512-col chunk is contiguous
    xtb = nc.declare_dram_parameter("xtb", [128, MC, KD, 512], BF16, False)
    xlo = nc.declare_dram_parameter("xlo", [128, MC, KD, 512], BF16, False)
    # n-major policy l1 weights: [e, p, n, k, c] = W[e, k*128+p, n*128+c]
    wp0 = nc.declare_dram_parameter("wp0", [E, 128, KD, KD, 128], BF16, False)
    wp1 = nc.declare_dram_parameter("wp1", [E, 128, KD, H], BF16, False)
    wmu = nc.declare_dram_parameter("wmu", [E, 128, KD, A], BF16, False)
    bp0 = nc.declare_dram_parameter("bp0", [E, 128, KD], F32, False)
    bp1 = nc.declare_dram_parameter("bp1", [E, 128, KD], F32, False)
    bmu = nc.declare_dram_parameter("bmu", [E, 128, A], F32, False)
    wv0 = nc.declare_dram_parameter("wv0", [E, 128, KD, HV], BF16, False)
    wv1 = nc.declare_dram_parameter("wv1", [E, 128, KH, HV], BF16, False)
    wv = nc.declare_dram_parameter("wv", [E, 128, KH], BF16, False)
    bv0 = nc.declare_dram_parameter("bv0", [E, 128, KH], F32, False)
    bv1 = nc.declare_dram_parameter("bv1", [E, 128, KH], F32, False)
    bvb = nc.declare_dram_parameter("bvb", [128, E], F32, False)
    ww0h = nc.declare_dram_parameter("ww0h", [128, KD, H], BF16, False)
    ww0l = nc.declare_dram_parameter("ww0l", [128, KD, H], BF16, False)
    ww1h = nc.declare_dram_parameter("ww1h", [128, KD, H], BF16, False)
    ww1l = nc.declare_dram_parameter("ww1l", [128, KD, H], BF16, False)
    wwoh = nc.declare_dram_parameter("wwoh", [128, KD, E], BF16, False)
    wwol = nc.declare_dram_parameter("wwol", [128, KD, E], BF16, False)
    bw0 = nc.declare_dram_parameter("bw0", [128, KD], F32, False)
    bw1 = nc.declare_dram_parameter("bw1", [128, KD], F32, False)
    bwo = nc.declare_dram_parameter("bwo", [128, E], F32, False)
    gmb = nc.declare_dram_parameter("gmb", [128, NT, E], F32, False)
    out_act = nc.declare_dram_parameter("actions", [E, NT // 4, 128, 4, A], F32, True)
    out_v = nc.declare_dram_parameter("v", [128, NT], F32, True)
    out_w = nc.declare_dram_parameter("w", [128, NT, E], F32, True)

    with tile.TileContext(nc) as tc:
        with (
            tc.tile_pool(name="persist", bufs=1) as persist,
            tc.tile_pool(name="xpool", bufs=1) as xpool,
            tc.tile_pool(name="gw", bufs=1) as gw,
            tc.tile_pool(name="gact", bufs=1) as gact,
            tc.tile_pool(name="ew", bufs=2) as ew,
            tc.tile_pool(name="eact", bufs=1) as eact,
            tc.tile_pool(name="eact2", bufs=2) as eact2,
            tc.tile_pool(name="stage", bufs=4) as stage,
            tc.tile_pool(name="smalls", bufs=4) as smalls,
            tc.tile_pool(name="pbig", bufs=5, space="PSUM") as pbig,
            tc.tile_pool(name="psmall", bufs=3, space="PSUM") as psmall,
        ):
            # x (bf16) first: the policy experts start as soon as the first
            # 512-column chunk lands; first-needed DMAs issued from separate
            # engines so descriptor generation runs in parallel
            xtb_sb = xpool.tile([128, MC, KD, 512], BF16)
            nc.sync.dma_start(xtb_sb[:, 0], xtb[:, 0])

            def load_policy_weights(e):
                eng = nc.gpsimd if e == 0 else nc.sync
                wp0_sb = ew.tile([128, KD, KD, 128], BF16, tag="wp0", name="wp0_sb")
                if e == 0:
                    for n in range(KD):
                        eng.dma_start(wp0_sb[:, n], wp0[e, :, n])
                else:
                    eng.dma_start(wp0_sb[:], wp0[e])
                bp0_sb = ew.tile([128, KD], F32, tag="bp0", name="bp0_sb")
                eng.dma_start(bp0_sb[:], bp0[e])
                if e == 0:
                    # first-needed data is in flight; queue the rest behind it
                    for m in range(1, MC):
                        nc.sync.dma_start(xtb_sb[:, m], xtb[:, m])
                    eng = nc.scalar
                wp1_sb = ew.tile([128, KD, H], BF16, tag="wp1", name="wp1_sb")
                eng.dma_start(wp1_sb[:], wp1[e])
                wmu_sb = ew.tile([128, KD, A], BF16, tag="wmu", name="wmu_sb")
                eng.dma_start(wmu_sb[:], wmu[e])
                bp1_sb = ew.tile([128, KD], F32, tag="bp1", name="bp1_sb")
                eng.dma_start(bp1_sb[:], bp1[e])
                bmu_sb = ew.tile([128, A], F32, tag="bmu", name="bmu_sb")
                eng.dma_start(bmu_sb[:], bmu[e])
                return wp0_sb, wp1_sb, wmu_sb, bp0_sb, bp1_sb, bmu_sb

            pol_w0 = load_policy_weights(0)

            # aux + gating tiles (DMAs deferred into the policy loop so the
            # first expert's loads get the full DMA bandwidth)
            gumbel_sb = persist.tile([128, NT, E], F32)
            w_sb = persist.tile([128, NT, E], F32)
            mask_sb = persist.tile([128, NT, E], F32)
            vals_b = persist.tile([128, NT, E], F32)
            v_sb = persist.tile([128, NT], F32)
            bwo_bc = persist.tile([128, E], F32)
            bvb_bc = persist.tile([128, E], F32)
            xlo_sb = xpool.tile([128, MC, KD, 512], BF16)
            ww0h_sb = gw.tile([128, KD, H], BF16)
            ww0l_sb = gw.tile([128, KD, H], BF16)
            ww1h_sb = gw.tile([128, KD, H], BF16)
            ww1l_sb = gw.tile([128, KD, H], BF16)
            wwoh_sb = gw.tile([128, KD, E], BF16)
            wwol_sb = gw.tile([128, KD, E], BF16)
            bw0_sb = gw.tile([128, KD], F32)
            bw1_sb = gw.tile([128, KD], F32)

            def load_aux():
                nc.sync.dma_start(gumbel_sb[:], gmb[:])
                nc.sync.dma_start(bwo_bc[:], bwo[:])
                nc.sync.dma_start(bvb_bc[:], bvb[:])
                nc.sync.dma_start(xlo_sb[:], xlo[:])
                nc.sync.dma_start(ww0h_sb[:], ww0h[:])
                nc.sync.dma_start(ww0l_sb[:], ww0l[:])
                nc.sync.dma_start(ww1h_sb[:], ww1h[:])
                nc.sync.dma_start(ww1l_sb[:], ww1l[:])
                nc.sync.dma_start(wwoh_sb[:], wwoh[:])
                nc.sync.dma_start(wwol_sb[:], wwol[:])
                nc.sync.dma_start(bw0_sb[:], bw0[:])
                nc.sync.dma_start(bw1_sb[:], bw1[:])

            # ---------------- expert policy nets (bf16) ----------------
            def policy_expert(e, weights):
                wp0_sb, wp1_sb, wmu_sb, bp0_sb, bp1_sb, bmu_sb = weights
                h1 = eact2.tile([128, KD, BL], BF16, tag="h1", name="h1")
                for n in range(KD):
                    for m in range(MC):
                        ps = pbig.tile([128, 512], F32, tag="big", name="ps")
                        for k in range(KD):
                            nc.tensor.matmul(
                                ps[:],
                                wp0_sb[:, n, k, :],
                                xtb_sb[:, m, k, :],
                                start=(k == 0),
                                stop=(k == KD - 1),
                            )
                        nc.scalar.activation(
                            h1[:, n, m * 512 : (m + 1) * 512],
                            ps[:],
                            AF.Relu,
                            bias=bp0_sb[:, n : n + 1],
                        )
                h2 = eact.tile([128, KD, BL], BF16, tag="h2", name="h2")
                for n in range(KD):
                    for m in range(MC):
                        ps = pbig.tile([128, 512], F32, tag="big", name="ps")
                        for k in range(KD):
                            nc.tensor.matmul(
                                ps[:],
                                wp1_sb[:, k, n * 128 : (n + 1) * 128],
                                h1[:, k, m * 512 : (m + 1) * 512],
                                start=(k == 0),
                                stop=(k == KD - 1),
                            )
                        nc.scalar.activation(
                            h2[:, n, m * 512 : (m + 1) * 512],
                            ps[:],
                            AF.Relu,
                            bias=bp1_sb[:, n : n + 1],
                        )
                # policy head: 4 batch tiles packed per PSUM bank
                for tq in range(NT // 4):
                    psA = psmall.tile([128, 4, A], F32, tag="small", name="psA")
                    for tt in range(4):
                        t = tq * 4 + tt
                        for k in range(KD):
                            nc.tensor.matmul(
                                psA[:, tt, :],
                                h2[:, k, t * 128 : (t + 1) * 128],
                                wmu_sb[:, k, :],
                                start=(tt == 0 and k == 0),
                                stop=(tt == 3 and k == KD - 1),
                            )
                    a_pre = smalls.tile([128, 4, A], F32, tag="a_pre", name="a_pre")
                    nc.vector.tensor_tensor(
                        a_pre[:],
                        psA[:],
                        bmu_sb[:, None, :].broadcast_to([128, 4, A]),
                        op=ALU.add,
                    )
                    a_st = stage.tile([128, 4, A], F32, tag="a_st", name="a_st")
                    nc.scalar.activation(a_st[:], a_pre[:], AF.Tanh)
                    nc.sync.dma_start(out_act[e, tq], a_st[:])

            policy_expert(0, pol_w0)
            for e in range(1, E):
                w_e = load_policy_weights(e)
                if e == 1:
                    load_aux()
                policy_expert(e, w_e)

            # ---------------- gating network (bf16x3 split precision) ----
            for mb in range(MC):
                ms = mb * 512
                u1 = gact.tile([128, KD, 512], F32, tag="u1", name="u1")
                for n in range(KD):
                    ps = pbig.tile([128, 512], F32, tag="big", name="ps")
                    nt_sl = slice(n * 128, (n + 1) * 128)
                    for k in range(KD):
                        nc.tensor.matmul(
                            ps[:], ww0h_sb[:, k, nt_sl],
                            xtb_sb[:, mb, k, :],
                            start=(k == 0), stop=False,
                        )
                    for k in range(KD):
                        nc.tensor.matmul(
                            ps[:], ww0l_sb[:, k, nt_sl],
                            xtb_sb[:, mb, k, :],
                            start=False, stop=False,
                        )
                    for k in range(KD):
                        nc.tensor.matmul(
                            ps[:], ww0h_sb[:, k, nt_sl],
                            xlo_sb[:, mb, k, :],
                            start=False, stop=(k == KD - 1),
                        )
                    nc.scalar.activation(
                        u1[:, n, :], ps[:], AF.Relu, bias=bw0_sb[:, n : n + 1]
                    )
                    if n == 0:
                        u1h = gact.tile([128, KD, 512], BF16, tag="u1h", name="u1h")
                        u1l = gact.tile([128, KD, 512], BF16, tag="u1l", name="u1l")
                    nc.vector.tensor_copy(u1h[:, n, :], u1[:, n, :])
                    nc.vector.tensor_sub(u1l[:, n, :], u1[:, n, :], u1h[:, n, :])
                u2 = gact.tile([128, KD, 512], F32, tag="u2", name="u2")
                for n in range(KD):
                    ps = pbig.tile([128, 512], F32, tag="big", name="ps")
                    nt_sl = slice(n * 128, (n + 1) * 128)
                    for k in range(KD):
                        nc.tensor.matmul(
                            ps[:], ww1h_sb[:, k, nt_sl], u1h[:, k, :],
                            start=(k == 0), stop=False,
                        )
                    for k in range(KD):
                        nc.tensor.matmul(
                            ps[:], ww1l_sb[:, k, nt_sl], u1h[:, k, :],
                            start=False, stop=False,
                        )
                    for k in range(KD):
                        nc.tensor.matmul(
                            ps[:], ww1h_sb[:, k, nt_sl], u1l[:, k, :],
                            start=False, stop=(k == KD - 1),
                        )
                    nc.scalar.activation(
                        u2[:, n, :], ps[:], AF.Relu, bias=bw1_sb[:, n : n + 1]
                    )
                    if n == 0:
                        u2h = gact.tile([128, KD, 512], BF16, tag="u2h", name="u2h")
                        u2l = gact.tile([128, KD, 512], BF16, tag="u2l", name="u2l")
                    nc.vector.tensor_copy(u2h[:, n, :], u2[:, n, :])
                    nc.vector.tensor_sub(u2l[:, n, :], u2[:, n, :], u2h[:, n, :])
                # logits (bf16x3) + softmax per 128-row tile
                for bt in range(4):
                    t = mb * 4 + bt
                    bt_sl = slice(bt * 128, (bt + 1) * 128)
                    ps8 = psmall.tile([128, E], F32, tag="small", name="ps8")
                    for k in range(KD):
                        nc.tensor.matmul(
                            ps8[:], u2h[:, k, bt_sl], wwoh_sb[:, k, :],
                            start=(k == 0), stop=False,
                        )
                    for k in range(KD):
                        nc.tensor.matmul(
                            ps8[:], u2h[:, k, bt_sl], wwol_sb[:, k, :],
                            start=False, stop=False,
                        )
                    for k in range(KD):
                        nc.tensor.matmul(
                            ps8[:], u2l[:, k, bt_sl], wwoh_sb[:, k, :],
                            start=False, stop=(k == KD - 1),
                        )
                    zl = smalls.tile([128, E], F32, tag="zl", name="zl")
                    nc.vector.tensor_add(zl[:], ps8[:], bwo_bc[:])
                    mx = smalls.tile([128, 1], F32, tag="mx", name="mx")
                    nc.vector.reduce_max(mx[:], zl[:], axis=AX.X)
                    negmx = smalls.tile([128, 1], F32, tag="negmx", name="negmx")
                    nc.vector.tensor_scalar_mul(negmx[:], mx[:], -1.0)
                    we = smalls.tile([128, E], F32, tag="we", name="we")
                    nc.scalar.activation(we[:], zl[:], AF.Exp, bias=negmx[:])
                    sm = smalls.tile([128, 1], F32, tag="sm", name="sm")
                    nc.vector.reduce_sum(sm[:], we[:], axis=AX.X)
                    rc = smalls.tile([128, 1], F32, tag="rc", name="rc")
                    nc.vector.reciprocal(rc[:], sm[:])
                    nc.vector.tensor_scalar_mul(w_sb[:, t, :], we[:], rc[:])
            nc.sync.dma_start(out_w[:], w_sb[:])

            # gumbel-argmax mask (batched; overlaps the value experts)
            z_sb = persist.tile([128, NT, E], F32)
            nc.scalar.activation(z_sb[:], w_sb[:], AF.Ln)
            nc.vector.tensor_add(z_sb[:], z_sb[:], gumbel_sb[:], )
            zmax = persist.tile([128, NT], F32)
            nc.vector.reduce_max(zmax[:], z_sb[:], axis=AX.X)
            nc.vector.tensor_tensor(
                mask_sb[:],
                z_sb[:],
                zmax[:, :, None].broadcast_to([128, NT, E]),
                op=ALU.is_ge,
            )
            # ---------------- expert value nets (bf16) ----------------
            for e in range(E):
                wv0_sb = ew.tile([128, KD, HV], BF16, tag="wv0", name="wv0_sb")
                nc.sync.dma_start(wv0_sb[:], wv0[e])
                wv1_sb = ew.tile([128, KH, HV], BF16, tag="wv1", name="wv1_sb")
                nc.sync.dma_start(wv1_sb[:], wv1[e])
                wv_sb = ew.tile([128, KH], BF16, tag="wv", name="wv_sb")
                nc.sync.dma_start(wv_sb[:], wv[e])
                bv0_sb = ew.tile([128, KH], F32, tag="bv0", name="bv0_sb")
                nc.sync.dma_start(bv0_sb[:], bv0[e])
                bv1_sb = ew.tile([128, KH], F32, tag="bv1", name="bv1_sb")
                nc.sync.dma_start(bv1_sb[:], bv1[e])

                g1 = eact2.tile([128, KH, BL], BF16, tag="g1", name="g1")
                for n in range(KH):
                    for m in range(MC):
                        ps = pbig.tile([128, 512], F32, tag="big", name="ps")
                        for k in range(KD):
                            nc.tensor.matmul(
                                ps[:],
                                wv0_sb[:, k, n * 128 : (n + 1) * 128],
                                xtb_sb[:, m, k, :],
                                start=(k == 0),
                                stop=(k == KD - 1),
                            )
                        if (n * MC + m) % 2 == 0:
                            nc.vector.tensor_scalar(
                                g1[:, n, m * 512 : (m + 1) * 512],
                                ps[:],
                                bv0_sb[:, n : n + 1],
                                0.0,
                                ALU.add,
                                op1=ALU.max,
                            )
                        else:
                            nc.scalar.activation(
                                g1[:, n, m * 512 : (m + 1) * 512],
                                ps[:],
                                AF.Relu,
                                bias=bv0_sb[:, n : n + 1],
                            )
                g2 = eact.tile([128, KH, BL], BF16, tag="g2", name="g2")
                for n in range(KH):
                    for m in range(MC):
                        ps = pbig.tile([128, 512], F32, tag="big", name="ps")
                        for k in range(KH):
                            nc.tensor.matmul(
                                ps[:],
                                wv1_sb[:, k, n * 128 : (n + 1) * 128],
                                g1[:, k, m * 512 : (m + 1) * 512],
                                start=(k == 0),
                                stop=(k == KH - 1),
                            )
                        if (n * MC + m) % 2 == 0:
                            nc.vector.tensor_scalar(
                                g2[:, n, m * 512 : (m + 1) * 512],
                                ps[:],
                                bv1_sb[:, n : n + 1],
                                0.0,
                                ALU.add,
                                op1=ALU.max,
                            )
                        else:
                            nc.scalar.activation(
                                g2[:, n, m * 512 : (m + 1) * 512],
                                ps[:],
                                AF.Relu,
                                bias=bv1_sb[:, n : n + 1],
                            )
                # value head: all 16 batch-tile columns in one PSUM bank
                psv = psmall.tile([128, NT], F32, tag="small", name="psv")
                for t in range(NT):
                    for k in range(KH):
                        nc.tensor.matmul(
                            psv[:, t : t + 1],
                            g2[:, k, t * 128 : (t + 1) * 128],
                            wv_sb[:, k : k + 1],
                            start=(t == 0 and k == 0),
                            stop=(t == NT - 1 and k == KH - 1),
                        )
                nc.vector.tensor_scalar_add(
                    vals_b[:, :, e], psv[:], bvb_bc[:, e : e + 1]
                )

            # ---------------- select v = sum_e mask_e * vals_e ------------
            vsel = persist.tile([128, NT, E], F32)
            nc.vector.tensor_mul(vsel[:], mask_sb[:], vals_b[:])
            nc.vector.reduce_sum(v_sb[:], vsel[:], axis=AX.X)
            nc.sync.dma_start(out_v[:], v_sb[:])

    nc.compile()
    return nc


# ------------------------- host-side wrapper -------------------------


def _prep_w(wmat, ks, npdt):
    # [K, N] -> [p, ks, n] with K = ks*128
    n = wmat.shape[1]
    return np.ascontiguousarray(
        wmat.reshape(ks, 128, n).transpose(1, 0, 2)
    ).astype(npdt)


def _prep_b(bvec, ks):
    return np.ascontiguousarray(bvec.reshape(ks, 128).T).astype(np.float32)


def _bcast(bvec):
    return np.ascontiguousarray(
        np.broadcast_to(np.asarray(bvec, dtype=np.float32)[None, :], (128, len(bvec)))
    )


def _gumbel():
    import jax
    import jax.numpy as jnp

    try:
        dev = jax.devices("cpu")[0]
        with jax.default_device(dev):
            g = jax.random.gumbel(jax.random.key(42), (B, E), jnp.float32)
            return np.asarray(g)
    except Exception:
        g = jax.random.gumbel(jax.random.key(42), (B, E), jnp.float32)
        return np.asarray(g)


def _split_hi_lo(w):
    w = np.asarray(w, dtype=np.float32)
    hi = w.astype(NPBF16)
    lo = (w - hi.astype(np.float32)).astype(NPBF16)
    return hi, lo


def kernel(x, Wp0, bp0, Wp1, bp1, Wmu, bmu, Wv0, bv0, Wv1, bv1, Wv, bv,
           Ww0, bw0, Ww1, bw1, Wwo, bwo):
    global _CACHED_NC, LAST_EXEC_NS

    x = np.asarray(x, dtype=np.float32)
    gumbel = _gumbel()

    ww0h, ww0l = _split_hi_lo(Ww0)
    ww1h, ww1l = _split_hi_lo(Ww1)
    wwoh_, wwol_ = _split_hi_lo(Wwo)

    common = {
        "wp0": np.stack(
            [
                np.ascontiguousarray(
                    np.asarray(Wp0[e], dtype=np.float32)
                    .reshape(KD, 128, KD, 128)
                    .transpose(1, 2, 0, 3)
                ).astype(NPBF16)
                for e in range(E)
            ]
        ),
        "wp1": np.stack([_prep_w(np.asarray(Wp1[e]), KD, NPBF16) for e in range(E)]),
        "wmu": np.stack([_prep_w(np.asarray(Wmu[e]), KD, NPBF16) for e in range(E)]),
        "bp0": np.stack([_prep_b(np.asarray(bp0[e]), KD) for e in range(E)]),
        "bp1": np.stack([_prep_b(np.asarray(bp1[e]), KD) for e in range(E)]),
        "bmu": np.ascontiguousarray(
            np.broadcast_to(np.asarray(bmu, dtype=np.float32)[:, None, :], (E, 128, A))
        ),
        "wv0": np.stack([_prep_w(np.asarray(Wv0[e]), KD, NPBF16) for e in range(E)]),
        "wv1": np.stack([_prep_w(np.asarray(Wv1[e]), KH, NPBF16) for e in range(E)]),
        "wv": np.stack(
            [_prep_w(np.asarray(Wv[e]), KH, NPBF16)[:, :, 0] for e in range(E)]
        ),
        "bv0": np.stack([_prep_b(np.asarray(bv0[e]), KH) for e in range(E)]),
        "bv1": np.stack([_prep_b(np.asarray(bv1[e]), KH) for e in range(E)]),
        "bvb": _bcast(np.asarray(bv, dtype=np.float32).reshape(E)),
        "ww0h": _prep_w(ww0h.astype(np.float32), KD, NPBF16),
        "ww0l": _prep_w(ww0l.astype(np.float32), KD, NPBF16),
        "ww1h": _prep_w(ww1h.astype(np.float32), KD, NPBF16),
        "ww1l": _prep_w(ww1l.astype(np.float32), KD, NPBF16),
        "wwoh": _prep_w(wwoh_.astype(np.float32), KD, NPBF16),
        "wwol": _prep_w(wwol_.astype(np.float32), KD, NPBF16),
        "bw0": _prep_b(np.asarray(bw0), KD),
        "bw1": _prep_b(np.asarray(bw1), KD),
        "bwo": _bcast(np.asarray(bwo, dtype=np.float32)),
    }

    in_maps = []
    for c in range(NCORES):
        xs = x[c * BL : (c + 1) * BL]  # [BL, D]
        xT = np.ascontiguousarray(xs.T)  # [D, BL]
        # [p, m, k, c] = xT[k*128+p, m*512+c]
        xtf = np.ascontiguousarray(
            xT.reshape(KD, 128, MC, 512).transpose(1, 2, 0, 3)
        ).astype(np.float32)
        xhi = xtf.astype(NPBF16)
        xlo_ = (xtf - xhi.astype(np.float32)).astype(NPBF16)
        gs = gumbel[c * BL : (c + 1) * BL]  # [BL, E]
        gtile = np.ascontiguousarray(
            gs.reshape(NT, 128, E).transpose(1, 0, 2)
        ).astype(np.float32)
        m = dict(common)
        m["xtb"] = xhi
        m["xlo"] = xlo_
        m["gmb"] = gtile
        in_maps.append(m)

    if _CACHED_NC is None:
        _CACHED_NC = _build()
    nc = _CACHED_NC

    trace = os.environ.get("KERNEL_TRACE", "0") == "1"
    if trace:
        import concourse.bass_utils as _bu

        _bu.upload_artifacts = lambda d: d
        try:
            import antenv.axon_hooks  # noqa: F401
        except ImportError:
            import types

            import antenv
            from trn_agent_boot.trn_boot import _ntff_profile_via_ctypes

            _hook = _ntff_profile_via_ctypes("/opt/axon/libaxon_pjrt.so")
            mod = types.ModuleType("antenv.axon_hooks")
            mod._hook = _hook
            mod.get_axon_ntff_profile_hook = lambda: mod._hook
            mod.set_axon_ntff_profile_hook = lambda h: setattr(mod, "_hook", h)
            sys.modules["antenv.axon_hooks"] = mod
            antenv.axon_hooks = mod

    res = run_bass_kernel_spmd(nc, in_maps, list(range(NCORES)), trace=trace)
    LAST_EXEC_NS = res.exec_time_ns

    act_parts, v_parts, w_parts = [], [], []
    for c in range(NCORES):
        r = res.results[c]
        act_parts.append(
            np.asarray(r["actions"]).transpose(0, 1, 3, 2, 4).reshape(E, BL, A)
        )
        v_parts.append(np.asarray(r["v"]).T.reshape(BL, 1))
        w_parts.append(np.asarray(r["w"]).transpose(1, 0, 2).reshape(BL, E))
    actions = np.concatenate(act_parts, axis=1)
    v = np.concatenate(v_parts, axis=0)
    w = np.concatenate(w_parts, axis=0)
    return actions, v, w
